# revision 1
# baseline (speedup 1.0000x reference)
"""Distributed Trainium2 kernel for nn_ActionEmbeddingModel.

Reference computation (B=4096, DC=1024, A=20000, C=128, H=1024):
    h         = relu(context @ w1 + b1)          # [B, H]
    ctx_score = h @ w2[:H]                       # [B]
    act_score = emb @ w2[H:]                     # [A]
    out[b, a] = ctx_score[b] + act_score[a] + b2 # [B, A]

Sharding (8 cores): data-parallel over the batch for context/h/ctx_score.
emb is sharded over actions for an AllGather of act_score; additionally
each core receives its own + two neighboring emb shards and computes those
act rows locally, so the first three output chunks depend only on local
work — the AllGather latency (collective entry + cross-core launch skew)
hides behind them. Device chunk j holds global action block (i+j) % 8;
the host un-rotates the column blocks when assembling the full output.

The [B/8, A] output shard is generated PE-free: act rows are partition-
broadcast on GpSimd and per-batch-row scores are added as per-partition
scalars on DVE/ACT, so the output phase is purely DMA-bound (its floor is
the 41 MB/core output write at ~358 GB/s HBM per-core bandwidth).

Matmuls run in float32r (fp32 bits, single-pass PE streaming, ~1.5e-4
rel err). Host-side prep only reorders memory; all FLOPs run on device.
"""

import numpy as np

import concourse.bass as bass
import concourse.mybir as mybir
from concourse import bacc
import concourse.tile as tile
from concourse.tile import TileContext
from concourse.bass_utils import run_bass_kernel_spmd

# Problem shape (hardcoded per harness contract).
B, DC, A, C, H = 4096, 1024, 20000, 128, 1024
N_CORES = 8
B_SH = B // N_CORES        # 512 batch rows per core
A_SH = A // N_CORES        # 2500 actions per block / emb shard
P = 128                    # partitions
KT = DC // P               # 8 contraction tiles for fc1
HT = H // P                # 8 hidden tiles
BT = B_SH // P             # 4 batch chunks of 128 rows
MM_N = 500                 # matmul free-dim chunk (<=512, even for f32r)
N_LOC = 3                  # act blocks computed locally (own + 2 neighbors)
F32 = mybir.dt.float32
F32R = mybir.dt.float32r

_CACHED_NC = None


def _build():
    nc = bacc.Bacc(num_devices=N_CORES)

    ctx_pp = nc.declare_dram_parameter("ctx_pp", [P, KT, B_SH], F32R, isOutput=False)
    w1_pp = nc.declare_dram_parameter("w1_pp", [HT, P, KT, P], F32R, isOutput=False)
    b1c = nc.declare_dram_parameter("b1c", [P, HT], F32, isOutput=False)
    w2h = nc.declare_dram_parameter("w2h", [P, HT], F32R, isOutput=False)
    w2c = nc.declare_dram_parameter("w2c", [P, 1], F32R, isOutput=False)
    b2 = nc.declare_dram_parameter("b2", [1, 1], F32, isOutput=False)
    one1 = nc.declare_dram_parameter("one1", [1, 1], F32, isOutput=False)
    # Per-core emb shards: index 0 = own shard (rank i), 1/2 = ranks i+1, i+2.
    embs = [
        nc.declare_dram_parameter(f"embT{j}", [C, A_SH], F32R, isOutput=False)
        for j in range(N_LOC)
    ]
    out_ext = nc.declare_dram_parameter("out", [B_SH, A], F32, isOutput=True)

    # Collective bounce buffers (collectives can't touch I/O tensors).
    ag_in = nc.dram_tensor("ag_in", [A_SH], F32)
    ag_out = nc.dram_tensor("ag_out", [A], F32, addr_space="Shared")

    relu = mybir.ActivationFunctionType.Relu
    ident = mybir.ActivationFunctionType.Identity

    with TileContext(nc, num_cores=N_CORES) as tc:
        with (
            tc.tile_pool(name="persist", bufs=1) as persist,
            tc.tile_pool(name="psum_h", bufs=4, space="PSUM") as pp,
            tc.tile_pool(name="psum_v", bufs=2, space="PSUM") as pp1,
            tc.tile_pool(name="psum_tr", bufs=1, space="PSUM") as trp,
        ):
            ctx_col = persist.tile([P, BT], F32, tag="ctx_col")

            # ---- small input DMAs first (b1 gates the relus) ----
            emb_sbs = []
            for j in range(N_LOC):
                e = persist.tile([C, A_SH], F32R, tag=f"emb{j}")
                emb_sbs.append(e)
            w2c_sb = persist.tile([P, 1], F32R, tag="w2c")
            nc.sync.dma_start(out=w2c_sb[:, :], in_=w2c[:, :])
            b2_sb = persist.tile([1, 1], F32, tag="b2")
            nc.sync.dma_start(out=b2_sb[:, :], in_=b2[:, :])
            b1_sb = persist.tile([P, HT], F32, tag="b1")
            nc.sync.dma_start(out=b1_sb[:, :], in_=b1c[:, :])
            w2h_sb = persist.tile([P, HT], F32R, tag="w2h")
            nc.sync.dma_start(out=w2h_sb[:, :], in_=w2h[:, :])
            one_sb = persist.tile([1, 1], F32, tag="one1")
            nc.sync.dma_start(out=one_sb[:, :], in_=one1[:, :])

            # ---- hT = relu(w1.T @ ctx.T + b1) and ctx_col; w1/ctx in a
            # ---- scoped pool released before the output tiles need SBUF ----
            with tc.tile_pool(name="fc1_pool", bufs=1) as fc1p:
                ctx_sb = fc1p.tile([P, KT * B_SH], F32R, tag="ctx")
                nc.sync.dma_start(
                    out=ctx_sb[:, :].rearrange("p (kt n) -> p kt n", kt=KT),
                    in_=ctx_pp[:, :, :],
                )
                nc.sync.dma_start(out=emb_sbs[0][:, :], in_=embs[0][:, :])
                w1_sb = fc1p.tile([P, HT * KT * P], F32R, tag="w1")
                for hb in range(HT):
                    nc.sync.dma_start(
                        out=w1_sb[
                            :, hb * KT * P:(hb + 1) * KT * P
                        ].rearrange("p (kt c) -> p kt c", kt=KT),
                        in_=w1_pp[hb, :, :, :],
                    )

                ht_tiles = []
                for ht in range(HT):
                    ps = pp.tile([P, B_SH], F32, tag="h_ps")
                    for kt in range(KT):
                        base = ht * KT * P + kt * P
                        mm = nc.tensor.matmul(
                            ps[:, :],
                            w1_sb[:, base:base + P],
                            ctx_sb[:, kt * B_SH:(kt + 1) * B_SH],
                            start=(kt == 0),
                            stop=(kt == KT - 1),
                        )
                        last_ht_mm = mm
                    hts = fc1p.tile([P, B_SH], F32R, tag=f"ht{ht}")
                    nc.scalar.activation(
                        hts[:, :], ps[:, :], relu, bias=b1_sb[:, ht:ht + 1]
                    )
                    ht_tiles.append(hts)
                    if ht == 0:
                        act_rows, last_mm = _emit_act0(
                            nc, tc, persist, pp1, emb_sbs, w2c_sb, b2_sb,
                            b1_sb, ag_in, ag_out,
                        )

                # ---- ctx_score row then transpose to ctx_col [128, BT] ----
                psc = pp1.tile([1, B_SH], F32, tag="act_ps")
                for ht in range(HT):
                    mm = nc.tensor.matmul(
                        psc[:, :],
                        w2h_sb[:, ht:ht + 1],
                        ht_tiles[ht][:, :],
                        start=(ht == 0),
                        stop=(ht == HT - 1),
                    )
                    if ht == 0:
                        tile.add_dep_helper(
                            mm.ins, last_mm.ins, sync=False,
                            reason="PE: ctx matvec after act0",
                        )
                ctx_row = persist.tile([1, B_SH], F32, tag="ctx_row")
                ctx_row_cp = nc.vector.tensor_copy(ctx_row[:, :], psc[:, :])
                last_tr_cp = None
                for bs in range(BT):
                    pst = trp.tile([P, 1], F32, tag="tr_ps")
                    nc.tensor.matmul(
                        pst[:, :],
                        ctx_row[0:1, bs * P:(bs + 1) * P],
                        one_sb[0:1, 0:1],
                        start=True,
                        stop=True,
                    )
                    last_tr_cp = nc.scalar.copy(ctx_col[:, bs:bs + 1], pst[:, :])

                # ---- neighbor act rows (needed from ~chunk-1 time on) ----
                for j in range(1, N_LOC):
                    nc.sync.dma_start(out=emb_sbs[j][:, :], in_=embs[j][:, :])
                    ar = persist.tile([1, A_SH], F32, tag=f"act{j}")
                    for at in range(A_SH // MM_N):
                        ps = pp1.tile([1, MM_N], F32, tag="act_ps")
                        mm = nc.tensor.matmul(
                            ps[:, :],
                            w2c_sb[:, :],
                            emb_sbs[j][:, at * MM_N:(at + 1) * MM_N],
                            start=True,
                            stop=True,
                        )
                        nc.scalar.add(
                            ar[:, at * MM_N:(at + 1) * MM_N],
                            ps[:, :],
                            b2_sb[0:1, 0:1],
                        )
                    act_rows.append(ar)

            # ---- output: device chunk j = global action block (pid+j)%8.
            # ---- Chunks 0..N_LOC-1 use local act rows; the rest read the
            # ---- AllGather at a dynamic (partition-id dependent) offset ----
            pid = nc.partition_id()
            with (
                tc.tile_pool(name="outp", bufs=5) as outp,
                tc.tile_pool(name="abcp", bufs=3) as abcp,
                tc.tile_pool(name="arowp", bufs=2) as arowp,
            ):
                for j in range(N_CORES):
                    if j < N_LOC:
                        src_row = act_rows[j]
                    else:
                        src_row = arowp.tile([1, A_SH], F32, tag="arow")
                        off = ((pid + j) % N_CORES) * A_SH
                        nc.gpsimd.dma_start(
                            out=src_row[:, :],
                            in_=ag_out[None, bass.ds(off, A_SH)],
                        )
                    act_bc = abcp.tile([P, A_SH], F32, tag="abc")
                    nc.gpsimd.partition_broadcast(act_bc[:, :], src_row[0:1, :])
                    for bs in range(BT):
                        o_sb = outp.tile([P, A_SH], F32, tag="osb")
                        if (j * BT + bs) % 2:
                            add_o = nc.scalar.activation(
                                o_sb[:, :], act_bc[:, :], ident,
                                bias=ctx_col[:, bs:bs + 1],
                            )
                            if j == 0 and bs == 1:
                                tile.add_dep_helper(
                                    add_o.ins, last_tr_cp.ins, sync=False,
                                    reason="ACT: out adds after tr copies",
                                )
                        else:
                            add_o = nc.vector.tensor_scalar_add(
                                o_sb[:, :], act_bc[:, :], ctx_col[:, bs:bs + 1]
                            )
                            if j == 0 and bs == 0:
                                tile.add_dep_helper(
                                    add_o.ins, ctx_row_cp.ins, sync=False,
                                    reason="DVE: out adds after ctx_row copy",
                                )
                        nc.sync.dma_start(
                            out=out_ext[
                                bs * P:(bs + 1) * P, j * A_SH:(j + 1) * A_SH
                            ],
                            in_=o_sb[:, :],
                        )
    nc.finalize()
    return nc


def _emit_act0(nc, tc, persist, pp1, emb_sbs, w2c_sb, b2_sb, b1_sb, ag_in, ag_out):
    """Own-shard act row + AllGather trigger, emitted right after hT group 0
    so the collective fires ~20 us earlier than if act0 queued behind all
    of fc1 on the PE."""
    import concourse.mybir as mybir
    MM = MM_N
    # GpSimd library warm-up so the first real bcast is hot.
    warm = persist.tile([P, 8], F32, tag="warm")
    nc.gpsimd.partition_broadcast(warm[:, :], b1_sb[0:1, 0:8])
    act_rows = []
    last_mm = None
    ar = persist.tile([1, A_SH], F32, tag="act0")
    for at in range(A_SH // MM):
        ps = pp1.tile([1, MM], F32, tag="act_ps")
        last_mm = nc.tensor.matmul(
            ps[:, :],
            w2c_sb[:, :],
            emb_sbs[0][:, at * MM:(at + 1) * MM],
            start=True,
            stop=True,
        )
        nc.scalar.add(
            ar[:, at * MM:(at + 1) * MM],
            ps[:, :],
            b2_sb[0:1, 0:1],
        )
    act_rows.append(ar)
    nc.gpsimd.dma_start(out=ag_in[None, :], in_=ar[0:1, :])
    nc.gpsimd.collective_compute(
        "AllGather",
        mybir.AluOpType.bypass,
        replica_groups=[list(range(N_CORES))],
        ins=[ag_in[:]],
        outs=[ag_out[:]],
    )
    return act_rows, last_mm


def _get_nc():
    global _CACHED_NC
    if _CACHED_NC is None:
        _CACHED_NC = _build()
    return _CACHED_NC


def _in_maps(context, w1, b1, emb, w2, b2):
    context = np.asarray(context, dtype=np.float32)
    w1 = np.asarray(w1, dtype=np.float32)
    b1 = np.asarray(b1, dtype=np.float32)
    emb = np.asarray(emb, dtype=np.float32)
    w2 = np.asarray(w2, dtype=np.float32)
    b2 = np.asarray(b2, dtype=np.float32)

    # w1_pp[hb, p, kt, c] = w1[kt*P + p, hb*P + c]
    w1_pp = np.ascontiguousarray(
        w1.reshape(KT, P, HT, P).transpose(2, 1, 0, 3)
    )
    b1c = np.ascontiguousarray(b1.reshape(HT, P).T)
    w2h = np.ascontiguousarray(w2[:H].reshape(HT, P).T)
    w2c = np.ascontiguousarray(w2[H:].reshape(P, 1))
    b2m = b2.reshape(1, 1)
    one1 = np.ones((1, 1), dtype=np.float32)
    emb_sh = [
        np.ascontiguousarray(emb[r * A_SH:(r + 1) * A_SH].T)
        for r in range(N_CORES)
    ]

    maps = []
    for i in range(N_CORES):
        ctx_sh = context[i * B_SH:(i + 1) * B_SH]
        # ctx_pp[p, kt, n] = context[n, kt*P + p]
        ctx_pp = np.ascontiguousarray(
            ctx_sh.T.reshape(KT, P, B_SH).transpose(1, 0, 2)
        )
        m = {
            "ctx_pp": ctx_pp,
            "w1_pp": w1_pp,
            "b1c": b1c,
            "w2h": w2h,
            "w2c": w2c,
            "b2": b2m,
            "one1": one1,
        }
        for j in range(N_LOC):
            m[f"embT{j}"] = emb_sh[(i + j) % N_CORES]
        maps.append(m)
    return maps


def kernel(context, w1, b1, emb, w2, b2, _trace=False, **_trace_kwargs):
    nc = _get_nc()
    maps = _in_maps(context, w1, b1, emb, w2, b2)
    res = run_bass_kernel_spmd(
        nc, maps, core_ids=list(range(N_CORES)), trace=_trace, **_trace_kwargs
    )
    out = np.empty((B, A), dtype=np.float32)
    for i in range(N_CORES):
        dev = res.results[i]["out"]
        for j in range(N_CORES):
            blk = (i + j) % N_CORES
            out[
                i * B_SH:(i + 1) * B_SH, blk * A_SH:(blk + 1) * A_SH
            ] = dev[:, j * A_SH:(j + 1) * A_SH]
    if _trace:
        return out, res
    return out



# revision 3
# speedup vs baseline: 1.5038x; 1.5038x over previous
"""Distributed Trainium2 kernel for nn_ActionEmbeddingModel.

Reference computation (B=4096, DC=1024, A=20000, C=128, H=1024):
    h         = relu(context @ w1 + b1)          # [B, H]
    ctx_score = h @ w2[:H]                       # [B]
    act_score = emb @ w2[H:]                     # [A]
    out[b, a] = ctx_score[b] + act_score[a] + b2 # [B, A]

Sharding (8 cores): pure data-parallel over the batch. emb and all weights
are replicated so every core computes all 20000 act scores locally; there
are NO collectives (the previous AllGather cost a ~38 us cross-core
barrier + ~25 us gather latency).

The whole device data path runs in bf16 (the rel-err gate is 2e-2; bf16
costs ~4e-3), halving the dominant HBM cost: the [512, 20000] per-core
output shard is written as bf16 (20.5 MB vs 41 MB) and up-cast to f32 on
the host. Input reads are bf16 too (5.2 MB/core). Total HBM traffic is
~28.8 MB/core -> ~82 us floor at ~350 GB/s effective.

Structure per core:
  - inputs stream in on the ACT HWDGE ring (emb chunk 0, ctx, w1 by
    h-block, emb chunks 1-7); output tiles stream out on the SP ring so
    reads and writes interleave at SDMA packet granularity.
  - fc1 runs batch-128-chunk-major so ctx_score for rows 0-127 is ready
    ~12 us in and output DMAs start while later inputs still load.
  - ctx_score needs no transpose: the h tiles [h,b] are reused as the
    matmul STATIONARY operand with w2h [128,1] moving, so the per-batch
    column [128(b),1] lands directly in PSUM.
  - act rows are matvec'd per 2500-wide chunk, partition-broadcast on
    GpSimd, and added to the per-partition ctx scalars on DVE/ACT.
"""

import numpy as np
import ml_dtypes

import concourse.bass as bass
import concourse.mybir as mybir
from concourse import bacc
from concourse.tile import TileContext
from concourse.bass_utils import run_bass_kernel_spmd

# Problem shape (hardcoded per harness contract).
B, DC, A, C, H = 4096, 1024, 20000, 128, 1024
N_CORES = 8
B_SH = B // N_CORES        # 512 batch rows per core
P = 128                    # partitions
KT = DC // P               # 8 contraction tiles for fc1
HT = H // P                # 8 hidden tiles
BT = B_SH // P             # 4 batch chunks of 128 rows
AC = 2500                  # action chunk width (8 chunks)
NC_A = A // AC             # 8 action chunks
MM_N = 500                 # matvec free-dim sub-chunk (psum-bank sized)
F32 = mybir.dt.float32
BF16 = mybir.dt.bfloat16
BF16_NP = ml_dtypes.bfloat16

_CACHED_NC = None


def _build():
    nc = bacc.Bacc(num_devices=N_CORES)

    ctx_pp = nc.declare_dram_parameter("ctx_pp", [P, KT, B_SH], BF16, isOutput=False)
    w1_pp = nc.declare_dram_parameter("w1_pp", [HT, P, KT, P], BF16, isOutput=False)
    b1c = nc.declare_dram_parameter("b1c", [P, HT], F32, isOutput=False)
    w2h = nc.declare_dram_parameter("w2h", [P, HT], BF16, isOutput=False)
    w2c = nc.declare_dram_parameter("w2c", [P, 1], BF16, isOutput=False)
    b2c = nc.declare_dram_parameter("b2c", [P, 1], F32, isOutput=False)
    embT = nc.declare_dram_parameter("embT", [C, A], BF16, isOutput=False)
    out_ext = nc.declare_dram_parameter("out", [B_SH, A], BF16, isOutput=True)

    relu = mybir.ActivationFunctionType.Relu
    ident = mybir.ActivationFunctionType.Identity

    with TileContext(nc, num_cores=N_CORES) as tc:
        with (
            tc.tile_pool(name="persist", bufs=1) as persist,
            tc.tile_pool(name="hts", bufs=10) as hp,
            tc.tile_pool(name="arow", bufs=3) as arp,
            tc.tile_pool(name="outp", bufs=6) as outp,
            tc.tile_pool(name="psum_f", bufs=4, space="PSUM") as ppf,
            tc.tile_pool(name="psum_v", bufs=2, space="PSUM") as ppv,
            tc.tile_pool(name="psum_c", bufs=2, space="PSUM") as ppc,
        ):
            # ---- input DMAs, all on the ACT (scalar) HWDGE ring, in
            # ---- priority order: smalls, emb chunk 0, ctx, w1, emb 1-7
            w2c_sb = persist.tile([P, 1], BF16, tag="w2c")
            nc.scalar.dma_start(out=w2c_sb[:, :], in_=w2c[:, :])
            b2_sb = persist.tile([P, 1], F32, tag="b2c")
            nc.scalar.dma_start(out=b2_sb[:, :], in_=b2c[:, :])
            b1_sb = persist.tile([P, HT], F32, tag="b1")
            nc.scalar.dma_start(out=b1_sb[:, :], in_=b1c[:, :])
            w2h_sb = persist.tile([P, HT], BF16, tag="w2h")
            nc.scalar.dma_start(out=w2h_sb[:, :], in_=w2h[:, :])

            emb_sbs = []
            for c in range(NC_A):
                e = persist.tile([C, AC], BF16, tag=f"emb{c}")
                emb_sbs.append(e)
            nc.scalar.dma_start(out=emb_sbs[0][:, :], in_=embT[:, 0:AC])

            ctx_sb = persist.tile([P, KT * B_SH], BF16, tag="ctx")
            nc.scalar.dma_start(
                out=ctx_sb[:, :].rearrange("p (kt n) -> p kt n", kt=KT),
                in_=ctx_pp[:, :, :],
            )
            w1_sbs = []
            for hb in range(HT):
                w = persist.tile([P, KT * P], BF16, tag=f"w1_{hb}")
                nc.scalar.dma_start(
                    out=w[:, :].rearrange("p (kt c) -> p kt c", kt=KT),
                    in_=w1_pp[hb, :, :, :],
                )
                w1_sbs.append(w)
            for c in range(1, NC_A):
                nc.scalar.dma_start(
                    out=emb_sbs[c][:, :], in_=embT[:, c * AC:(c + 1) * AC]
                )

            # GpSimd library warm-up so the first real bcast is hot.
            warm = persist.tile([P, 8], F32, tag="warm")
            nc.gpsimd.partition_broadcast(warm[:, :], b1_sb[0:1, 0:8])

            ctx_col = persist.tile([P, BT], F32, tag="ctx_col")
            act_bcs = []
            for c in range(NC_A):
                abc = persist.tile([P, AC], BF16, tag=f"abc{c}")
                act_bcs.append(abc)

            def emit_act_chunk(c):
                """act row chunk c: matvec emb.T @ w2c, then partition-bcast."""
                ar = arp.tile([1, AC], BF16, tag="arow")
                for s in range(AC // MM_N):
                    ps = ppv.tile([1, MM_N], F32, tag="mv_ps")
                    nc.tensor.matmul(
                        ps[:, :],
                        w2c_sb[:, :],
                        emb_sbs[c][:, s * MM_N:(s + 1) * MM_N],
                        start=True,
                        stop=True,
                    )
                    nc.vector.tensor_copy(ar[:, s * MM_N:(s + 1) * MM_N], ps[:, :])
                nc.gpsimd.partition_broadcast(act_bcs[c][:, :], ar[0:1, :])

            def emit_fc1_bs(bs):
                """h tiles for batch rows bs*128..+128, then ctx_col[:, bs]."""
                ht_tiles = []
                for ht in range(HT):
                    ps = ppf.tile([P, P], F32, tag="h_ps")
                    for kt in range(KT):
                        nc.tensor.matmul(
                            ps[:, :],
                            w1_sbs[ht][:, kt * P:(kt + 1) * P],
                            ctx_sb[:, kt * B_SH + bs * P: kt * B_SH + (bs + 1) * P],
                            start=(kt == 0),
                            stop=(kt == KT - 1),
                        )
                    hts = hp.tile([P, P], BF16, tag="ht")
                    nc.scalar.activation(
                        hts[:, :], ps[:, :], relu, bias=b1_sb[:, ht:ht + 1]
                    )
                    ht_tiles.append(hts)
                # ctx_col[b] = sum_h h[h, b] * w2h[h]: h tiles are the
                # STATIONARY operand so the result lands as [128(b), 1].
                pst = ppc.tile([P, 1], F32, tag="cs_ps")
                for ht in range(HT):
                    nc.tensor.matmul(
                        pst[:, :],
                        ht_tiles[ht][:, :],
                        w2h_sb[:, ht:ht + 1],
                        start=(ht == 0),
                        stop=(ht == HT - 1),
                    )
                nc.scalar.add(ctx_col[:, bs:bs + 1], pst[:, :], b2_sb[:, 0:1])

            # PE stream order: interleave act chunks with fc1 batch chunks
            # to match DMA arrival order (emb c0, ctx, w1, emb c1-7).
            emit_act_chunk(0)
            emit_fc1_bs(0)
            emit_fc1_bs(1)
            emit_act_chunk(1)
            emit_fc1_bs(2)
            emit_act_chunk(2)
            emit_act_chunk(3)
            emit_fc1_bs(3)
            for c in range(4, NC_A):
                emit_act_chunk(c)

            # ---- output tiles: out[bs*128+p, c*2500+a] =
            # ----   act_bc[c][p?, a] + ctx_col[p, bs]; adds split DVE/ACT.
            # ---- DMAs go out on the SP (sync) ring.
            for bs in range(BT):
                for c in range(NC_A):
                    o_sb = outp.tile([P, AC], BF16, tag="osb")
                    if (bs * NC_A + c) % 3 == 2:
                        nc.scalar.activation(
                            o_sb[:, :], act_bcs[c][:, :], ident,
                            bias=ctx_col[:, bs:bs + 1],
                        )
                    else:
                        nc.vector.tensor_scalar_add(
                            o_sb[:, :], act_bcs[c][:, :], ctx_col[:, bs:bs + 1]
                        )
                    nc.sync.dma_start(
                        out=out_ext[
                            bs * P:(bs + 1) * P, c * AC:(c + 1) * AC
                        ],
                        in_=o_sb[:, :],
                    )
    nc.finalize()
    return nc


def _get_nc():
    global _CACHED_NC
    if _CACHED_NC is None:
        _CACHED_NC = _build()
    return _CACHED_NC


def _in_maps(context, w1, b1, emb, w2, b2):
    context = np.asarray(context, dtype=np.float32)
    w1 = np.asarray(w1, dtype=np.float32)
    b1 = np.asarray(b1, dtype=np.float32)
    emb = np.asarray(emb, dtype=np.float32)
    w2 = np.asarray(w2, dtype=np.float32)
    b2 = np.asarray(b2, dtype=np.float32)

    # w1_pp[hb, p, kt, c] = w1[kt*P + p, hb*P + c]
    w1_pp = np.ascontiguousarray(
        w1.reshape(KT, P, HT, P).transpose(2, 1, 0, 3)
    ).astype(BF16_NP)
    b1c = np.ascontiguousarray(b1.reshape(HT, P).T)
    w2h = np.ascontiguousarray(w2[:H].reshape(HT, P).T).astype(BF16_NP)
    w2c = np.ascontiguousarray(w2[H:].reshape(P, 1)).astype(BF16_NP)
    b2c = np.broadcast_to(b2.reshape(1, 1), (P, 1)).astype(np.float32).copy()
    embT = np.ascontiguousarray(emb.T).astype(BF16_NP)

    maps = []
    for i in range(N_CORES):
        ctx_sh = context[i * B_SH:(i + 1) * B_SH]
        # ctx_pp[p, kt, n] = context[n, kt*P + p]
        ctx_pp = np.ascontiguousarray(
            ctx_sh.T.reshape(KT, P, B_SH).transpose(1, 0, 2)
        ).astype(BF16_NP)
        maps.append({
            "ctx_pp": ctx_pp,
            "w1_pp": w1_pp,
            "b1c": b1c,
            "w2h": w2h,
            "w2c": w2c,
            "b2c": b2c,
            "embT": embT,
        })
    return maps


def kernel(context, w1, b1, emb, w2, b2, _trace=False, **_trace_kwargs):
    nc = _get_nc()
    maps = _in_maps(context, w1, b1, emb, w2, b2)
    res = run_bass_kernel_spmd(
        nc, maps, core_ids=list(range(N_CORES)), trace=_trace, **_trace_kwargs
    )
    out = np.empty((B, A), dtype=np.float32)
    for i in range(N_CORES):
        out[i * B_SH:(i + 1) * B_SH, :] = res.results[i]["out"].astype(np.float32)
    if _trace:
        return out, res
    return out


# revision 4
# speedup vs baseline: 1.8486x; 1.2293x over previous
"""Distributed Trainium2 kernel for nn_ActionEmbeddingModel.

Reference computation (B=4096, DC=1024, A=20000, C=128, H=1024):
    h         = relu(context @ w1 + b1)          # [B, H]
    ctx_score = h @ w2[:H]                       # [B]
    act_score = emb @ w2[H:]                     # [A]
    out[b, a] = ctx_score[b] + act_score[a] + b2 # [B, A]

Sharding (8 cores): pure data-parallel over the batch; emb and weights are
replicated so every core computes all act scores locally - NO collectives
(a cross-core barrier + AllGather measured ~60 us of latency/skew).

Precision: rel-err gate is 2e-2. Output and emb path run in bf16; the fc1
operands (ctx, w1) are fp8 e4m3 with w1 pre-scaled by 16 on the host (to
clear the e4m3 subnormal range) and descaled by the relu's scale=1/16.
Measured rel err ~1e-2. This cuts per-core HBM traffic to
~27 MB (0.5 ctx + 1.05 w1 + 5.1 emb reads + 20.5 MB output write),
~70 us at the ~390 GB/s effective per-core rate.

Key structure (all per core):
  - Inputs stream on the GpSimd SWDGE ring (so neither ACT nor SP pay
    trigger costs), outputs on the SP HWDGE ring.
  - act_score is matvec'd with a column-REPLICATED w2c stationary
    [128(C) x 128], so each [128, 500] matmul lands in PSUM already
    partition-broadcast; four of them fill a [128, 2000] PSUM region that
    one CAST converts to a bf16 act_bc tile. No GpSimd broadcast, no
    [1,500] row staging.
  - fc1 runs in two batch-pair passes ([128, 256] moving operand: half
    the LDWEIGHTS of 128-wide, twice-earlier ctx_score than 512-wide).
    ctx_score needs no transpose: the h tiles [h, b] are the STATIONARY
    operand with w2h [128,1] moving, so [128(b), 1] lands in PSUM.
  - out tiles [128, 2000] = act_bc + ctx_col scalar on DVE (3/4) and
    ACT (1/4), 40 tiles, DMA'd as 512 KB writes.
"""

import numpy as np
import ml_dtypes

import concourse.bass as bass
import concourse.mybir as mybir
from concourse import bacc
from concourse.tile import TileContext
from concourse.bass_utils import run_bass_kernel_spmd

# Problem shape (hardcoded per harness contract).
B, DC, A, C, H = 4096, 1024, 20000, 128, 1024
N_CORES = 8
B_SH = B // N_CORES        # 512 batch rows per core
P = 128                    # partitions
KT = DC // P               # 8 contraction tiles for fc1
HT = H // P                # 8 hidden tiles
NPAIR = 2                  # two batch pairs of 256 rows
PW = 256                   # pair width (fc1 moving free dim)
AC = 2000                  # action chunk width
NC_A = A // AC             # 10 action chunks
MM_N = 500                 # matvec free-dim sub-chunk (psum-bank sized)
W1_SCALE = 16.0            # fp8 w1 pre-scale (cleared by relu scale=1/16)
F32 = mybir.dt.float32
BF16 = mybir.dt.bfloat16
FP8 = mybir.dt.float8e4
BF16_NP = ml_dtypes.bfloat16
FP8_NP = ml_dtypes.float8_e4m3fn

_CACHED_NC = None


def _build():
    nc = bacc.Bacc(num_devices=N_CORES)

    ctx_pp = nc.declare_dram_parameter("ctx_pp", [P, KT, B_SH], FP8, isOutput=False)
    w1_pp = nc.declare_dram_parameter("w1_pp", [HT, P, KT, P], FP8, isOutput=False)
    b1c = nc.declare_dram_parameter("b1c", [P, HT], F32, isOutput=False)
    w2h = nc.declare_dram_parameter("w2h", [P, HT], BF16, isOutput=False)
    w2cb = nc.declare_dram_parameter("w2cb", [C, P], BF16, isOutput=False)
    b2c = nc.declare_dram_parameter("b2c", [P, 1], F32, isOutput=False)
    embT = nc.declare_dram_parameter("embT", [C, A], BF16, isOutput=False)
    out_ext = nc.declare_dram_parameter("out", [B_SH, A], BF16, isOutput=True)

    relu = mybir.ActivationFunctionType.Relu
    ident = mybir.ActivationFunctionType.Identity

    with TileContext(nc, num_cores=N_CORES) as tc:
        with (
            tc.tile_pool(name="persist", bufs=1) as persist,
            tc.tile_pool(name="hts", bufs=10) as hp,
            tc.tile_pool(name="outp", bufs=6) as outp,
            tc.tile_pool(name="psum_f", bufs=2, space="PSUM") as ppf,
            tc.tile_pool(name="psum_v", bufs=1, space="PSUM") as ppv,
            tc.tile_pool(name="psum_c", bufs=2, space="PSUM") as ppc,
        ):
            # ---- input DMAs on the GpSimd SWDGE ring, in priority order:
            # ---- smalls, emb chunk 0, ctx, w1, emb chunks 1..9
            w2cb_sb = persist.tile([C, P], BF16, tag="w2cb")
            nc.gpsimd.dma_start(out=w2cb_sb[:, :], in_=w2cb[:, :])
            b2_sb = persist.tile([P, 1], F32, tag="b2c")
            nc.gpsimd.dma_start(out=b2_sb[:, :], in_=b2c[:, :])
            b1_sb = persist.tile([P, HT], F32, tag="b1")
            nc.gpsimd.dma_start(out=b1_sb[:, :], in_=b1c[:, :])
            w2h_sb = persist.tile([P, HT], BF16, tag="w2h")
            nc.gpsimd.dma_start(out=w2h_sb[:, :], in_=w2h[:, :])

            emb_sbs = []
            for c in range(NC_A):
                e = persist.tile([C, AC], BF16, tag=f"emb{c}")
                emb_sbs.append(e)
            nc.gpsimd.dma_start(out=emb_sbs[0][:, :], in_=embT[:, 0:AC])

            ctx_sb = persist.tile([P, KT * B_SH], FP8, tag="ctx")
            nc.gpsimd.dma_start(
                out=ctx_sb[:, :].rearrange("p (kt n) -> p kt n", kt=KT),
                in_=ctx_pp[:, :, :],
            )
            w1_sbs = []
            for hb in range(HT):
                w = persist.tile([P, KT * P], FP8, tag=f"w1_{hb}")
                nc.gpsimd.dma_start(
                    out=w[:, :].rearrange("p (kt c) -> p kt c", kt=KT),
                    in_=w1_pp[hb, :, :, :],
                )
                w1_sbs.append(w)
            for c in range(1, NC_A):
                nc.gpsimd.dma_start(
                    out=emb_sbs[c][:, :], in_=embT[:, c * AC:(c + 1) * AC]
                )

            ctx_col = persist.tile([P, 2 * NPAIR], F32, tag="ctx_col")
            act_bcs = []
            for c in range(NC_A):
                abc = persist.tile([P, AC], BF16, tag=f"abc{c}")
                act_bcs.append(abc)

            def emit_act_chunk(c):
                """act chunk c: [128,2000] PSUM already partition-broadcast
                via the replicated-w2c stationary; one CAST to bf16 SBUF."""
                ps = ppv.tile([P, AC], F32, tag="mv_ps")
                for s in range(AC // MM_N):
                    nc.tensor.matmul(
                        ps[:, s * MM_N:(s + 1) * MM_N],
                        w2cb_sb[:, :],
                        emb_sbs[c][:, s * MM_N:(s + 1) * MM_N],
                        start=True,
                        stop=True,
                    )
                if c % 2 == 0:
                    nc.vector.tensor_copy(act_bcs[c][:, :], ps[:, :])
                else:
                    nc.scalar.copy(act_bcs[c][:, :], ps[:, :])

            def emit_fc1_pair(pair):
                """h tiles for batch rows pair*256..+256, then ctx_col for
                both 128-row halves (h tiles as stationary, w2h moving)."""
                ht_tiles = []
                for ht in range(HT):
                    ps = ppf.tile([P, PW], F32, tag="h_ps")
                    for kt in range(KT):
                        nc.tensor.matmul(
                            ps[:, :],
                            w1_sbs[ht][:, kt * P:(kt + 1) * P],
                            ctx_sb[:, kt * B_SH + pair * PW:
                                   kt * B_SH + (pair + 1) * PW],
                            start=(kt == 0),
                            stop=(kt == KT - 1),
                        )
                    hts = hp.tile([P, PW], BF16, tag="ht")
                    nc.scalar.activation(
                        hts[:, :], ps[:, :], relu,
                        bias=b1_sb[:, ht:ht + 1], scale=1.0 / W1_SCALE,
                    )
                    ht_tiles.append(hts)
                for half in range(2):
                    bs = 2 * pair + half
                    pst = ppc.tile([P, 1], F32, tag="cs_ps")
                    for ht in range(HT):
                        nc.tensor.matmul(
                            pst[:, :],
                            ht_tiles[ht][:, half * P:(half + 1) * P],
                            w2h_sb[:, ht:ht + 1],
                            start=(ht == 0),
                            stop=(ht == HT - 1),
                        )
                    nc.scalar.add(ctx_col[:, bs:bs + 1], pst[:, :], b2_sb[:, 0:1])

            # PE stream order tuned to DMA arrival order: act chunk 0 first
            # (emb c0 is the first big input), fc1 pair 0, then act chunks
            # as they land, fc1 pair 1 after chunk 5.
            emit_act_chunk(0)
            emit_fc1_pair(0)
            for c in range(1, 6):
                emit_act_chunk(c)
            emit_fc1_pair(1)
            for c in range(6, NC_A):
                emit_act_chunk(c)

            # ---- output tiles [128,2000]: act_bc[c] + ctx_col[:, bs],
            # ---- 3/4 on DVE, 1/4 on ACT; DMA out on the SP ring.
            for bs in range(2 * NPAIR):
                for c in range(NC_A):
                    o_sb = outp.tile([P, AC], BF16, tag="osb")
                    if (bs * NC_A + c) % 4 == 3:
                        nc.scalar.activation(
                            o_sb[:, :], act_bcs[c][:, :], ident,
                            bias=ctx_col[:, bs:bs + 1],
                        )
                    else:
                        nc.vector.tensor_scalar_add(
                            o_sb[:, :], act_bcs[c][:, :], ctx_col[:, bs:bs + 1]
                        )
                    nc.sync.dma_start(
                        out=out_ext[
                            bs * P:(bs + 1) * P, c * AC:(c + 1) * AC
                        ],
                        in_=o_sb[:, :],
                    )
    nc.finalize()
    return nc


def _get_nc():
    global _CACHED_NC
    if _CACHED_NC is None:
        _CACHED_NC = _build()
    return _CACHED_NC


def _in_maps(context, w1, b1, emb, w2, b2):
    context = np.asarray(context, dtype=np.float32)
    w1 = np.asarray(w1, dtype=np.float32)
    b1 = np.asarray(b1, dtype=np.float32)
    emb = np.asarray(emb, dtype=np.float32)
    w2 = np.asarray(w2, dtype=np.float32)
    b2 = np.asarray(b2, dtype=np.float32)

    # w1_pp[hb, p, kt, c] = w1[kt*P + p, hb*P + c] (x16: fp8 subnormal dodge)
    w1_pp = np.ascontiguousarray(
        (w1 * W1_SCALE).reshape(KT, P, HT, P).transpose(2, 1, 0, 3)
    ).astype(FP8_NP)
    b1c = np.ascontiguousarray(b1.reshape(HT, P).T)
    w2h = np.ascontiguousarray(w2[:H].reshape(HT, P).T).astype(BF16_NP)
    # w2cb[k, p] = w2[H + k] for every p: replicated stationary so the
    # act matvec output is partition-broadcast for free.
    w2cb = np.ascontiguousarray(
        np.broadcast_to(w2[H:].reshape(C, 1), (C, P))
    ).astype(BF16_NP)
    b2c = np.broadcast_to(b2.reshape(1, 1), (P, 1)).astype(np.float32).copy()
    embT = np.ascontiguousarray(emb.T).astype(BF16_NP)

    maps = []
    for i in range(N_CORES):
        ctx_sh = context[i * B_SH:(i + 1) * B_SH]
        # ctx_pp[p, kt, n] = context[n, kt*P + p]
        ctx_pp = np.ascontiguousarray(
            ctx_sh.T.reshape(KT, P, B_SH).transpose(1, 0, 2)
        ).astype(FP8_NP)
        maps.append({
            "ctx_pp": ctx_pp,
            "w1_pp": w1_pp,
            "b1c": b1c,
            "w2h": w2h,
            "w2cb": w2cb,
            "b2c": b2c,
            "embT": embT,
        })
    return maps


def kernel(context, w1, b1, emb, w2, b2, _trace=False, **_trace_kwargs):
    nc = _get_nc()
    maps = _in_maps(context, w1, b1, emb, w2, b2)
    res = run_bass_kernel_spmd(
        nc, maps, core_ids=list(range(N_CORES)), trace=_trace, **_trace_kwargs
    )
    out = np.empty((B, A), dtype=np.float32)
    for i in range(N_CORES):
        out[i * B_SH:(i + 1) * B_SH, :] = res.results[i]["out"].astype(np.float32)
    if _trace:
        return out, res
    return out


# revision 10
# speedup vs baseline: 1.8541x; 1.0030x over previous
"""Distributed Trainium2 kernel for nn_ActionEmbeddingModel.

Reference computation (B=4096, DC=1024, A=20000, C=128, H=1024):
    h         = relu(context @ w1 + b1)          # [B, H]
    ctx_score = h @ w2[:H]                       # [B]
    act_score = emb @ w2[H:]                     # [A]
    out[b, a] = ctx_score[b] + act_score[a] + b2 # [B, A]

Sharding (8 cores): pure data-parallel over the batch; emb and weights are
replicated so every core computes all act scores locally - NO collectives
(a cross-core barrier + AllGather measured ~60 us of latency/skew).

The whole device data path runs in bf16 (rel-err gate 2e-2, measured
~3e-3): the [512, 20000] per-core output shard is written as bf16
(20.5 MB vs 41 MB f32) and up-cast on the host; inputs are bf16 reads
(5.2 MB). Total ~28.8 MB HBM traffic/core = ~72 us at the measured
~400 GB/s per-core DMA rate; everything else hides behind it.
(fp8 e4m3 fc1 was tried: rel err 2.5e-2 > gate. bf16 it is.)

Structure per core:
  - Inputs stream on the GpSimd SWDGE ring (emb c0-c1, ctx, w1, emb
    c2-19), tiny tables on the ACT ring, output tiles on the SP HWDGE
    ring - so no engine pays trigger costs on its critical path and
    reads/writes interleave at SDMA packet granularity.
  - act_score is matvec'd with a column-REPLICATED w2c stationary
    [128(C) x 128], so each [128, 512] matmul lands in PSUM already
    partition-broadcast. Chunks are 1024 wide (2 psum banks, double
    buffered); one CAST per chunk (DVE/ACT alternating) converts PSUM
    to a bf16 act_bc tile.
  - fc1 is a single [128, 512]-moving pass (64 matmuls; one LDWEIGHTS
    per matmul is the PE cost driver, so widest legal moving operand).
    ctx_score needs no transpose: h tiles [h, b] are the STATIONARY
    operand with w2h [128, 1] moving -> [128(b), 1] lands in PSUM.
  - out tiles [128, 4096] assembled from 4 chunk-adds each (DVE 3/4,
    ACT 1/4), DMA'd as 1 MB writes (20 per core).
"""

import numpy as np
import ml_dtypes

import concourse.bass as bass
import concourse.mybir as mybir
from concourse import bacc
from concourse.tile import TileContext
from concourse.bass_utils import run_bass_kernel_spmd

# Problem shape (hardcoded per harness contract).
B, DC, A, C, H = 4096, 1024, 20000, 128, 1024
N_CORES = 8
B_SH = B // N_CORES        # 512 batch rows per core
P = 128                    # partitions
KT = DC // P               # 8 contraction tiles for fc1
HT = H // P                # 8 hidden tiles
BT = B_SH // P             # 4 batch blocks of 128 rows
# Action chunks: 1024-wide (2 psum banks; every matvec matmul is 512-wide
# = exactly one bank, so accumulation never crosses banks). Last chunk 544.
A_W = [1024] * 19 + [544]
A_S = [1024 * i for i in range(20)]
NC_A = len(A_W)
MM_N = 512
# Output tiles: 4 chunks each -> [128, 4096] 1 MB DMAs (last 3616 wide).
O_W = [4096, 4096, 4096, 4096, 3616]
O_S = [4096 * i for i in range(5)]
NT_A = len(O_W)
F32 = mybir.dt.float32
BF16 = mybir.dt.bfloat16
BF16_NP = ml_dtypes.bfloat16

_CACHED_NC = None


def _build():
    nc = bacc.Bacc(num_devices=N_CORES)

    ctx_pp = nc.declare_dram_parameter("ctx_pp", [P, KT, B_SH], BF16, isOutput=False)
    w1_pp = nc.declare_dram_parameter("w1_pp", [HT, P, KT, P], BF16, isOutput=False)
    b1c = nc.declare_dram_parameter("b1c", [P, HT], F32, isOutput=False)
    w2h = nc.declare_dram_parameter("w2h", [P, HT], BF16, isOutput=False)
    w2cb = nc.declare_dram_parameter("w2cb", [C, P], BF16, isOutput=False)
    b2c = nc.declare_dram_parameter("b2c", [P, 1], F32, isOutput=False)
    embT = nc.declare_dram_parameter("embT", [C, A], BF16, isOutput=False)
    out_ext = nc.declare_dram_parameter("out", [B_SH, A], BF16, isOutput=True)

    relu = mybir.ActivationFunctionType.Relu
    ident = mybir.ActivationFunctionType.Identity

    with TileContext(nc, num_cores=N_CORES) as tc:
        with (
            tc.tile_pool(name="persist", bufs=1) as persist,
            tc.tile_pool(name="hts", bufs=9) as hp,
            tc.tile_pool(name="outp", bufs=6) as outp,
            tc.tile_pool(name="psum_f", bufs=2, space="PSUM") as ppf,
            tc.tile_pool(name="psum_v", bufs=2, space="PSUM") as ppv,
            tc.tile_pool(name="psum_c", bufs=2, space="PSUM") as ppc,
        ):
            # ---- tiny tables on the ACT HWDGE ring (keeps SWDGE streaming)
            w2cb_sb = persist.tile([C, P], BF16, tag="w2cb")
            nc.scalar.dma_start(out=w2cb_sb[:, :], in_=w2cb[:, :])
            b2_sb = persist.tile([P, 1], F32, tag="b2c")
            nc.scalar.dma_start(out=b2_sb[:, :], in_=b2c[:, :])
            b1_sb = persist.tile([P, HT], F32, tag="b1")
            nc.scalar.dma_start(out=b1_sb[:, :], in_=b1c[:, :])
            w2h_sb = persist.tile([P, HT], BF16, tag="w2h")
            nc.scalar.dma_start(out=w2h_sb[:, :], in_=w2h[:, :])

            # ---- bulk inputs on the GpSimd SWDGE ring, in priority order:
            # ---- emb chunks 0-1, ctx, w1 by h-block, emb chunks 2-19
            emb_sbs = []
            for c in range(NC_A):
                e = persist.tile([C, A_W[c]], BF16, tag=f"emb{c}")
                emb_sbs.append(e)
            for c in range(2):
                nc.gpsimd.dma_start(
                    out=emb_sbs[c][:, :], in_=embT[:, A_S[c]:A_S[c] + A_W[c]]
                )
            ctx_sb = persist.tile([P, KT * B_SH], BF16, tag="ctx")
            nc.gpsimd.dma_start(
                out=ctx_sb[:, :].rearrange("p (kt n) -> p kt n", kt=KT),
                in_=ctx_pp[:, :, :],
            )
            w1_sbs = []
            for hb in range(HT):
                w = persist.tile([P, KT * P], BF16, tag=f"w1_{hb}")
                nc.gpsimd.dma_start(
                    out=w[:, :].rearrange("p (kt c) -> p kt c", kt=KT),
                    in_=w1_pp[hb, :, :, :],
                )
                w1_sbs.append(w)
            for c in range(2, NC_A):
                nc.gpsimd.dma_start(
                    out=emb_sbs[c][:, :], in_=embT[:, A_S[c]:A_S[c] + A_W[c]]
                )

            ctx_col = persist.tile([P, BT], F32, tag="ctx_col")
            act_bcs = []
            for c in range(NC_A):
                abc = persist.tile([P, A_W[c]], BF16, tag=f"abc{c}")
                act_bcs.append(abc)

            def emit_act_chunk(c):
                """act chunk c: [128, A_W[c]] PSUM, already partition-
                broadcast via the replicated-w2c stationary; one CAST."""
                w = A_W[c]
                ps = ppv.tile([P, w], F32, tag="mv_ps")
                for off in range(0, w, MM_N):
                    sw = min(MM_N, w - off)
                    nc.tensor.matmul(
                        ps[:, off:off + sw],
                        w2cb_sb[:, :],
                        emb_sbs[c][:, off:off + sw],
                        start=True,
                        stop=True,
                    )
                if c % 2 == 0:
                    nc.vector.tensor_copy(act_bcs[c][:, :], ps[:, :])
                else:
                    nc.scalar.copy(act_bcs[c][:, :], ps[:, :])

            def emit_fc1():
                """Single-pass fc1 ([128,512] moving), then ctx_col for all
                4 batch blocks (h tiles as stationary, w2h moving)."""
                ht_tiles = []
                for ht in range(HT):
                    ps = ppf.tile([P, B_SH], F32, tag="h_ps")
                    for kt in range(KT):
                        nc.tensor.matmul(
                            ps[:, :],
                            w1_sbs[ht][:, kt * P:(kt + 1) * P],
                            ctx_sb[:, kt * B_SH:(kt + 1) * B_SH],
                            start=(kt == 0),
                            stop=(kt == KT - 1),
                        )
                    hts = hp.tile([P, B_SH], BF16, tag="ht")
                    nc.scalar.activation(
                        hts[:, :], ps[:, :], relu, bias=b1_sb[:, ht:ht + 1]
                    )
                    ht_tiles.append(hts)
                for bs in range(BT):
                    pst = ppc.tile([P, 1], F32, tag="cs_ps")
                    for ht in range(HT):
                        nc.tensor.matmul(
                            pst[:, :],
                            ht_tiles[ht][:, bs * P:(bs + 1) * P],
                            w2h_sb[:, ht:ht + 1],
                            start=(ht == 0),
                            stop=(ht == HT - 1),
                        )
                    nc.scalar.add(ctx_col[:, bs:bs + 1], pst[:, :], b2_sb[:, 0:1])

            # PE stream order matches DMA arrival order: act chunks 0-1
            # (emb lands first), fc1 (ctx+w1), act chunks 2-19 as they land.
            emit_act_chunk(0)
            emit_act_chunk(1)
            emit_fc1()
            for c in range(2, NC_A):
                emit_act_chunk(c)

            # ---- out tiles [128, O_W[t]] = 4 chunk-adds (act_bc[c] +
            # ---- ctx_col[:, bs]) on DVE (3/4) / ACT (1/4); SP-ring DMA.
            for bs in range(BT):
                for t in range(NT_A):
                    o_sb = outp.tile([P, O_W[t]], BF16, tag="osb")
                    for c in range(4 * t, min(4 * t + 4, NC_A)):
                        lo = A_S[c] - O_S[t]
                        dst = o_sb[:, lo:lo + A_W[c]]
                        if (bs * NC_A + c) % 4 == 3:
                            nc.scalar.activation(
                                dst, act_bcs[c][:, :], ident,
                                bias=ctx_col[:, bs:bs + 1],
                            )
                        else:
                            nc.vector.tensor_scalar_add(
                                dst, act_bcs[c][:, :], ctx_col[:, bs:bs + 1]
                            )
                    nc.sync.dma_start(
                        out=out_ext[
                            bs * P:(bs + 1) * P, O_S[t]:O_S[t] + O_W[t]
                        ],
                        in_=o_sb[:, :],
                    )
    nc.finalize()
    return nc


def _get_nc():
    global _CACHED_NC
    if _CACHED_NC is None:
        _CACHED_NC = _build()
    return _CACHED_NC


def _in_maps(context, w1, b1, emb, w2, b2):
    context = np.asarray(context, dtype=np.float32)
    w1 = np.asarray(w1, dtype=np.float32)
    b1 = np.asarray(b1, dtype=np.float32)
    emb = np.asarray(emb, dtype=np.float32)
    w2 = np.asarray(w2, dtype=np.float32)
    b2 = np.asarray(b2, dtype=np.float32)

    # w1_pp[hb, p, kt, c] = w1[kt*P + p, hb*P + c]
    w1_pp = np.ascontiguousarray(
        w1.reshape(KT, P, HT, P).transpose(2, 1, 0, 3)
    ).astype(BF16_NP)
    b1c = np.ascontiguousarray(b1.reshape(HT, P).T)
    w2h = np.ascontiguousarray(w2[:H].reshape(HT, P).T).astype(BF16_NP)
    # w2cb[k, p] = w2[H + k] for every p: replicated stationary so the
    # act matvec output is partition-broadcast for free.
    w2cb = np.ascontiguousarray(
        np.broadcast_to(w2[H:].reshape(C, 1), (C, P))
    ).astype(BF16_NP)
    b2c = np.broadcast_to(b2.reshape(1, 1), (P, 1)).astype(np.float32).copy()
    embT = np.ascontiguousarray(emb.T).astype(BF16_NP)

    maps = []
    for i in range(N_CORES):
        ctx_sh = context[i * B_SH:(i + 1) * B_SH]
        # ctx_pp[p, kt, n] = context[n, kt*P + p]
        ctx_pp = np.ascontiguousarray(
            ctx_sh.T.reshape(KT, P, B_SH).transpose(1, 0, 2)
        ).astype(BF16_NP)
        maps.append({
            "ctx_pp": ctx_pp,
            "w1_pp": w1_pp,
            "b1c": b1c,
            "w2h": w2h,
            "w2cb": w2cb,
            "b2c": b2c,
            "embT": embT,
        })
    return maps


def kernel(context, w1, b1, emb, w2, b2, _trace=False, **_trace_kwargs):
    nc = _get_nc()
    maps = _in_maps(context, w1, b1, emb, w2, b2)
    res = run_bass_kernel_spmd(
        nc, maps, core_ids=list(range(N_CORES)), trace=_trace, **_trace_kwargs
    )
    out = np.empty((B, A), dtype=np.float32)
    for i in range(N_CORES):
        out[i * B_SH:(i + 1) * B_SH, :] = res.results[i]["out"].astype(np.float32)
    if _trace:
        return out, res
    return out


# revision 12
# speedup vs baseline: 1.9073x; 1.0287x over previous
"""Distributed Trainium2 kernel for nn_ActionEmbeddingModel.

Reference computation (B=4096, DC=1024, A=20000, C=128, H=1024):
    h         = relu(context @ w1 + b1)          # [B, H]
    ctx_score = h @ w2[:H]                       # [B]
    act_score = emb @ w2[H:]                     # [A]
    out[b, a] = ctx_score[b] + act_score[a] + b2 # [B, A]

Sharding (8 cores): pure data-parallel over the batch; emb and weights are
replicated so every core computes all act scores locally - NO collectives
(a cross-core barrier + AllGather measured ~60 us of latency/skew).

The whole device data path runs in bf16 (rel-err gate 2e-2, measured
~3e-3): the [512, 20000] per-core output shard is written as bf16
(20.5 MB vs 41 MB f32) and up-cast on the host; inputs are bf16 reads
(5.2 MB). Total ~28.8 MB HBM traffic/core = ~72 us at the measured
~400 GB/s per-core DMA rate; everything else hides behind it.
(fp8 e4m3 fc1 was tried: rel err 2.5e-2 > gate. bf16 it is.)

Structure per core:
  - Inputs stream on the GpSimd SWDGE ring (emb c0-c1, ctx, w1, emb
    c2-19), tiny tables on the ACT ring, output tiles on the SP HWDGE
    ring - so no engine pays trigger costs on its critical path and
    reads/writes interleave at SDMA packet granularity.
  - act_score is matvec'd with a column-REPLICATED w2c stationary
    [128(C) x 128], so each [128, 512] matmul lands in PSUM already
    partition-broadcast. Chunks are 1024 wide (2 psum banks, double
    buffered); one CAST per chunk (DVE/ACT alternating) converts PSUM
    to a bf16 act_bc tile.
  - fc1 is a single [128, 512]-moving pass (64 matmuls; one LDWEIGHTS
    per matmul is the PE cost driver, so widest legal moving operand).
    ctx_score needs no transpose: h tiles [h, b] are the STATIONARY
    operand with w2h [128, 1] moving -> [128(b), 1] lands in PSUM.
  - out tiles [128, 4096] assembled from 4 chunk-adds each (DVE 3/4,
    ACT 1/4), DMA'd as 1 MB writes (20 per core).
"""

import numpy as np
import ml_dtypes

import concourse.bass as bass
import concourse.mybir as mybir
from concourse import bacc
from concourse.tile import TileContext
from concourse.bass_utils import run_bass_kernel_spmd

# Problem shape (hardcoded per harness contract).
B, DC, A, C, H = 4096, 1024, 20000, 128, 1024
N_CORES = 8
B_SH = B // N_CORES        # 512 batch rows per core
P = 128                    # partitions
KT = DC // P               # 8 contraction tiles for fc1
HT = H // P                # 8 hidden tiles
BT = B_SH // P             # 4 batch blocks of 128 rows
# Action chunks: 1024-wide (2 psum banks; every matvec matmul is 512-wide
# = exactly one bank, so accumulation never crosses banks). Last chunk 544.
A_W = [1024] * 19 + [544]
A_S = [1024 * i for i in range(20)]
NC_A = len(A_W)
MM_N = 512
# Output tiles: 4 chunks each -> [128, 4096] 1 MB DMAs (last 3616 wide).
O_W = [4096, 4096, 4096, 4096, 3616]
O_S = [4096 * i for i in range(5)]
NT_A = len(O_W)
F32 = mybir.dt.float32
BF16 = mybir.dt.bfloat16
BF16_NP = ml_dtypes.bfloat16

_CACHED_NC = None


def _build():
    nc = bacc.Bacc(num_devices=N_CORES)

    ctx_pp = nc.declare_dram_parameter("ctx_pp", [P, KT, B_SH], BF16, isOutput=False)
    w1_pp = nc.declare_dram_parameter("w1_pp", [HT, P, KT, P], BF16, isOutput=False)
    b1c = nc.declare_dram_parameter("b1c", [P, HT], F32, isOutput=False)
    w2h = nc.declare_dram_parameter("w2h", [P, HT], BF16, isOutput=False)
    w2cb = nc.declare_dram_parameter("w2cb", [C, P], BF16, isOutput=False)
    b2c = nc.declare_dram_parameter("b2c", [P, 1], F32, isOutput=False)
    embT = nc.declare_dram_parameter("embT", [C, A], BF16, isOutput=False)
    out_ext = nc.declare_dram_parameter("out", [B_SH, A], BF16, isOutput=True)

    relu = mybir.ActivationFunctionType.Relu
    ident = mybir.ActivationFunctionType.Identity

    with TileContext(nc, num_cores=N_CORES) as tc:
        with (
            tc.tile_pool(name="persist", bufs=1) as persist,
            tc.tile_pool(name="hts", bufs=9) as hp,
            tc.tile_pool(name="outp", bufs=7) as outp,
            tc.tile_pool(name="psum_f", bufs=2, space="PSUM") as ppf,
            tc.tile_pool(name="psum_v", bufs=2, space="PSUM") as ppv,
            tc.tile_pool(name="psum_c", bufs=2, space="PSUM") as ppc,
        ):
            # ---- tiny tables on the ACT HWDGE ring (keeps SWDGE streaming)
            w2cb_sb = persist.tile([C, P], BF16, tag="w2cb")
            nc.scalar.dma_start(out=w2cb_sb[:, :], in_=w2cb[:, :])
            b2_sb = persist.tile([P, 1], F32, tag="b2c")
            nc.scalar.dma_start(out=b2_sb[:, :], in_=b2c[:, :])
            b1_sb = persist.tile([P, HT], F32, tag="b1")
            nc.scalar.dma_start(out=b1_sb[:, :], in_=b1c[:, :])
            w2h_sb = persist.tile([P, HT], BF16, tag="w2h")
            nc.scalar.dma_start(out=w2h_sb[:, :], in_=w2h[:, :])

            # ---- bulk inputs on the GpSimd SWDGE ring, in priority order:
            # ---- emb chunks 0-1, ctx, w1 by h-block, emb chunks 2-19
            emb_sbs = []
            for c in range(NC_A):
                e = persist.tile([C, A_W[c]], BF16, tag=f"emb{c}")
                emb_sbs.append(e)
            for c in range(2):
                nc.gpsimd.dma_start(
                    out=emb_sbs[c][:, :], in_=embT[:, A_S[c]:A_S[c] + A_W[c]]
                )
            ctx_sb = persist.tile([P, KT * B_SH], BF16, tag="ctx")
            nc.gpsimd.dma_start(
                out=ctx_sb[:, :].rearrange("p (kt n) -> p kt n", kt=KT),
                in_=ctx_pp[:, :, :],
            )
            w1_sbs = []
            for hb in range(HT):
                w = persist.tile([P, KT * P], BF16, tag=f"w1_{hb}")
                nc.gpsimd.dma_start(
                    out=w[:, :].rearrange("p (kt c) -> p kt c", kt=KT),
                    in_=w1_pp[hb, :, :, :],
                )
                w1_sbs.append(w)
            for c in range(2, NC_A):
                nc.gpsimd.dma_start(
                    out=emb_sbs[c][:, :], in_=embT[:, A_S[c]:A_S[c] + A_W[c]]
                )

            ctx_col = persist.tile([P, BT], F32, tag="ctx_col")
            act_bcs = []
            for c in range(NC_A):
                abc = persist.tile([P, A_W[c]], BF16, tag=f"abc{c}")
                act_bcs.append(abc)

            def emit_act_chunk(c):
                """act chunk c: [128, A_W[c]] PSUM, already partition-
                broadcast via the replicated-w2c stationary; one CAST."""
                w = A_W[c]
                ps = ppv.tile([P, w], F32, tag="mv_ps")
                for off in range(0, w, MM_N):
                    sw = min(MM_N, w - off)
                    nc.tensor.matmul(
                        ps[:, off:off + sw],
                        w2cb_sb[:, :],
                        emb_sbs[c][:, off:off + sw],
                        start=True,
                        stop=True,
                    )
                if c % 2 == 0:
                    nc.vector.tensor_copy(act_bcs[c][:, :], ps[:, :])
                else:
                    nc.scalar.copy(act_bcs[c][:, :], ps[:, :])

            PW = 2 * P  # fc1 pair width: 256 batch rows per pass

            def emit_fc1_pair(pair, mv_chunks=()):
                """h tiles for batch rows pair*256..+256 ([128,256] moving:
                ctx_col for the first pair lands ~13 us earlier than a
                single 512-wide pass), then ctx_col for both 128-row halves
                (h tiles as stationary, w2h moving). Act-chunk matvecs from
                mv_chunks are interleaved between h-blocks so act tiles
                keep flowing while fc1 owns the PE."""
                mv_chunks = list(mv_chunks)
                ht_tiles = []
                for ht in range(HT):
                    ps = ppf.tile([P, PW], F32, tag="h_ps")
                    for kt in range(KT):
                        nc.tensor.matmul(
                            ps[:, :],
                            w1_sbs[ht][:, kt * P:(kt + 1) * P],
                            ctx_sb[:, kt * B_SH + pair * PW:
                                   kt * B_SH + (pair + 1) * PW],
                            start=(kt == 0),
                            stop=(kt == KT - 1),
                        )
                    hts = hp.tile([P, PW], BF16, tag="ht")
                    nc.scalar.activation(
                        hts[:, :], ps[:, :], relu, bias=b1_sb[:, ht:ht + 1]
                    )
                    ht_tiles.append(hts)
                    if mv_chunks:
                        emit_act_chunk(mv_chunks.pop(0))
                for half in range(2):
                    bs = 2 * pair + half
                    pst = ppc.tile([P, 1], F32, tag="cs_ps")
                    for ht in range(HT):
                        nc.tensor.matmul(
                            pst[:, :],
                            ht_tiles[ht][:, half * P:(half + 1) * P],
                            w2h_sb[:, ht:ht + 1],
                            start=(ht == 0),
                            stop=(ht == HT - 1),
                        )
                    nc.scalar.add(ctx_col[:, bs:bs + 1], pst[:, :], b2_sb[:, 0:1])

            # PE stream order matches DMA arrival order: act chunks 0-1
            # (emb lands first), fc1 pair 0 (ctx+w1), act chunks as they
            # land, fc1 pair 1 with later chunks woven between h-blocks.
            emit_act_chunk(0)
            emit_act_chunk(1)
            emit_fc1_pair(0)
            for c in range(2, 8):
                emit_act_chunk(c)
            emit_fc1_pair(1, mv_chunks=range(8, 16))
            for c in range(16, NC_A):
                emit_act_chunk(c)

            # ---- out tiles [128, O_W[t]] = 4 chunk-adds (act_bc[c] +
            # ---- ctx_col[:, bs]) on DVE (3/4) / ACT (1/4); SP-ring DMA.
            for bs in range(BT):
                for t in range(NT_A):
                    o_sb = outp.tile([P, O_W[t]], BF16, tag="osb")
                    for c in range(4 * t, min(4 * t + 4, NC_A)):
                        lo = A_S[c] - O_S[t]
                        dst = o_sb[:, lo:lo + A_W[c]]
                        if (bs * NC_A + c) % 4 == 3:
                            nc.scalar.activation(
                                dst, act_bcs[c][:, :], ident,
                                bias=ctx_col[:, bs:bs + 1],
                            )
                        else:
                            nc.vector.tensor_scalar_add(
                                dst, act_bcs[c][:, :], ctx_col[:, bs:bs + 1]
                            )
                    nc.sync.dma_start(
                        out=out_ext[
                            bs * P:(bs + 1) * P, O_S[t]:O_S[t] + O_W[t]
                        ],
                        in_=o_sb[:, :],
                    )
    nc.finalize()
    return nc


def _get_nc():
    global _CACHED_NC
    if _CACHED_NC is None:
        _CACHED_NC = _build()
    return _CACHED_NC


def _in_maps(context, w1, b1, emb, w2, b2):
    context = np.asarray(context, dtype=np.float32)
    w1 = np.asarray(w1, dtype=np.float32)
    b1 = np.asarray(b1, dtype=np.float32)
    emb = np.asarray(emb, dtype=np.float32)
    w2 = np.asarray(w2, dtype=np.float32)
    b2 = np.asarray(b2, dtype=np.float32)

    # w1_pp[hb, p, kt, c] = w1[kt*P + p, hb*P + c]
    w1_pp = np.ascontiguousarray(
        w1.reshape(KT, P, HT, P).transpose(2, 1, 0, 3)
    ).astype(BF16_NP)
    b1c = np.ascontiguousarray(b1.reshape(HT, P).T)
    w2h = np.ascontiguousarray(w2[:H].reshape(HT, P).T).astype(BF16_NP)
    # w2cb[k, p] = w2[H + k] for every p: replicated stationary so the
    # act matvec output is partition-broadcast for free.
    w2cb = np.ascontiguousarray(
        np.broadcast_to(w2[H:].reshape(C, 1), (C, P))
    ).astype(BF16_NP)
    b2c = np.broadcast_to(b2.reshape(1, 1), (P, 1)).astype(np.float32).copy()
    embT = np.ascontiguousarray(emb.T).astype(BF16_NP)

    maps = []
    for i in range(N_CORES):
        ctx_sh = context[i * B_SH:(i + 1) * B_SH]
        # ctx_pp[p, kt, n] = context[n, kt*P + p]
        ctx_pp = np.ascontiguousarray(
            ctx_sh.T.reshape(KT, P, B_SH).transpose(1, 0, 2)
        ).astype(BF16_NP)
        maps.append({
            "ctx_pp": ctx_pp,
            "w1_pp": w1_pp,
            "b1c": b1c,
            "w2h": w2h,
            "w2cb": w2cb,
            "b2c": b2c,
            "embT": embT,
        })
    return maps


def kernel(context, w1, b1, emb, w2, b2, _trace=False, **_trace_kwargs):
    nc = _get_nc()
    maps = _in_maps(context, w1, b1, emb, w2, b2)
    res = run_bass_kernel_spmd(
        nc, maps, core_ids=list(range(N_CORES)), trace=_trace, **_trace_kwargs
    )
    out = np.empty((B, A), dtype=np.float32)
    for i in range(N_CORES):
        out[i * B_SH:(i + 1) * B_SH, :] = res.results[i]["out"].astype(np.float32)
    if _trace:
        return out, res
    return out
